# revision 34
# baseline (speedup 1.0000x reference)
"""Binarized-MLP (BNN) kernel for Trainium2, data-parallel over batch on 8 cores.

Reference computation:
    h      = x @ sign(W1) + b1          x:[8192,4096] W1:[4096,512]
    logits = sign(h) @ sign(W2) + b2    W2:[512,10]
    out    = softmax(logits)            [8192,10]

Strategy (per core, batch shard of 1024 rows):
  - x is host-split as x = hi + lo/2^11 with hi = fp16(x) (11 significant
    bits) and lo = e4m3((x - hi) * 2^11). The hi pass runs as 256 normal
    fp16 matmuls (K=128 stationary tiles); the lo pass runs as 128 fp8
    DoubleRow matmuls (K=256 tiles, 2 MACs/cell/cycle). The lo scale 2^-11
    is folded into the stationary operand as sign(W1) * 2^-11 in e5m2.
    Total stream: 384 matmuls = 1.5x one bf16 copy, h accurate to ~2e-4.
  - Stream order: hi quads 0..7 back-to-back, then lo groups 0..3, then
    the bf16 second layer — exactly 2 PE dtype-mode transitions
    (fp16->fp8, fp8->bf16), each ~0.2-0.6us.
  - DMA rings: the steady stream (whi, xhi, wlo, xlo, output) rides the
    sync HWDGE ring, whose FIFO gives just-in-time delivery order.
    Startup pieces x00a/x01/x23 ride the scalar (ACT) ring and x00b the
    gpsimd SWDGE ring for parallel first-transfer latency. (Moving the
    steady stream onto the scalar ring silently corrupts data beyond ~3
    queued transfers; eager xlo prefetch on a second ring also steals
    early HBM bandwidth from the critical quad-1 path — both measured.)
  - Startup: N=128 fp16 warmup matmuls off a memset tile bridge the PE
    from ~7.4us until the first data lands ~11us, keeping the HAM
    activity window busy so the PE reaches 2.4GHz ~3.4us after the first
    warmup with no mid-ramp resets (a >150ns gap during the ramp resets
    the 4096-cycle window and costs ~2.8us).
  - PSUM: 8 banks [j][bc] of [128,512] accumulate all 48 matmuls per bank.
  - Phase 2: last lo group bank-major; all 8 sign() activations are
    emitted before any softmax work (strict-FIFO ACT queue). b2 - 64 is
    folded into the second matmul's accumulation group as a 5th matmul
    (stationary = ones/128 in bf16, moving = (b2-64) replicated), so ps2
    holds logits - 64 and softmax needs NO max-reduction: max|logit| ~
    6.7 sigma = 150 << 88+64, so exp(logit-64) cannot overflow fp32, and
    exp underflow of all 10 classes needs max < -23 (P ~ 1e-8/row).
    Per 4-bt chunk: one ACT Exp straight off PSUM -> segmented
    reduce_sum -> reciprocal, exp and 1/sum ship to the host which does
    the broadcast multiply (a dequant-like pointwise scale).
  - Output: packed [p, bt*11+c] f32 (10 exp values + sum per bt), one
    DMA on the sync ring; host multiplies and reorders to [1024,10].
"""

import numpy as np
import ml_dtypes

import concourse.bass as bass
import concourse.tile as tile
from concourse import mybir
from concourse.bass_utils import run_bass_kernel_spmd
from bass_rust import ScopedClock, VectorClock

_CLEAR_SEMS = True

E4 = mybir.dt.float8e4
E5 = mybir.dt.float8e5
FP16 = mybir.dt.float16
BF16 = mybir.dt.bfloat16
F32 = mybir.dt.float32
DRMODE = mybir.MatmulPerfMode.DoubleRow

B, F, H, C = 8192, 4096, 512, 10
NCORES = 8
BC = B // NCORES          # 1024 batch rows per core
NFT = F // 128            # 32 hi f-tiles
NHQ = NFT // 4            # 8 hi quads
NQ = F // 256             # 16 lo DR blocks
NLG = NQ // 4             # 4 lo groups
NJ = H // 128             # 4 j-tiles
NBC = BC // 512           # 2 moving chunks of 512
NBT = BC // 128           # 8 output b-tiles
LOSHIFT = 11              # lo scale 2^11
NWARM = 33                # N=128 warmup matmuls bridging DMA latency
SMXW = C + 1              # per-bt output: 10 exp values + 1 row sum


class _PatchedTileContext(tile.TileContext):
    """Workaround for the walrus build in this container only accepting one
    sem wait on a CTRL-type (Drain) instruction: spread the exit drain's
    per-proc waits across several drains with one wait each."""

    def _drain_and_barrier(self, tick_clock, wait_clock):
        gc = tick_clock.global_clock
        ticks = list(gc)
        nprocs = len(ticks)
        engines = [
            self.nc.sync,
            self.nc.gpsimd,
            self.nc.vector,
            self.nc.scalar,
            self.nc.tensor,
        ]
        k = 0
        for i, t in enumerate(ticks):
            if t == 0:
                continue
            partial = [0] * nprocs
            partial[i] = t
            inst = engines[k % len(engines)].nop()
            k += 1
            wait_clock.add_sem_waits(
                inst.ins, ScopedClock({None: VectorClock(partial)})
            )
        self.nc.sync.drain()

        self.nc.all_engine_barrier(sem_only=True)
        assert self.sems is not None
        popped = self.nc._tile_sem_poison_stack.pop()
        assert popped is self._sem_poison
        if _CLEAR_SEMS:
            self.nc.clear_and_free_semaphores(list(self.sems.allocated().values()))


def _split_waits_json(raw: bytes) -> bytes:
    """The walrus build in this container accepts at most ONE sem wait per
    instruction. Rewrite the serialized BIR: excess waits become standalone
    EventSemaphore wait instructions on the same engine immediately before
    the instruction."""
    import json as _json

    m = _json.loads(raw)
    ctr = 0
    for fn in m.get("functions", []):
        for bb in fn.get("blocks", []):
            insts = bb.get("instructions", [])
            new_insts = []
            for inst in insts:
                si = inst.get("sync_info")
                waits = si.get("on_wait") or [] if si else []
                if len(waits) > 1:
                    for w in waits[:-1]:
                        new_insts.append(
                            {
                                "debug": inst.get("debug", 0),
                                "engine": inst["engine"],
                                "ins": [],
                                "outs": [],
                                "name": f"WSPLIT-{ctr}",
                                "opcode": "EventSemaphore",
                                "sync_info": {"on_update": [], "on_wait": [w]},
                            }
                        )
                        ctr += 1
                    si["on_wait"] = [waits[-1]]
                new_insts.append(inst)
            bb["instructions"] = new_insts
    return _json.dumps(m).encode()


def _install_wait_splitter(nc: bass.Bass) -> None:
    orig = nc.to_json_bytes

    def patched():
        return _split_waits_json(orig())

    nc.to_json_bytes = patched


def build_kernel() -> bass.Bass:
    nc = bass.Bass()
    # hi stream: row hq*128+p, free [i=0..3][b]   (8KB / partition line)
    xhi = nc.dram_tensor("xhi", [NHQ * 128, 4 * BC], FP16, kind="ExternalInput")
    # hi weights: row hq*128+p, free [i=0..3][j*128+col]  (4KB / line)
    whi = nc.dram_tensor("whi", [NHQ * 128, 4 * H], FP16, kind="ExternalInput")
    # lo stream: row g*128+p, free [u=0..3][k][b]  (8KB / line)
    xlo = nc.dram_tensor("xlo", [NLG * 128, 4 * 2 * BC], E4, kind="ExternalInput")
    # lo weights: row g*128+p, free [u][j][k][col]  (4KB / line)
    wlo = nc.dram_tensor("wlo", [NLG * 128, 4 * NJ * 256], E5, kind="ExternalInput")
    # constants packed as one byte blob: b1 f32 [0:16) | sign(W2) bf16
    # [16:96) | (b2-64) bf16 [96:116)
    cblob = nc.dram_tensor("cblob", [128, 116], mybir.dt.uint8,
                           kind="ExternalInput")
    out = nc.dram_tensor("out", [128, NBC * 4 * SMXW], F32, kind="ExternalOutput")

    with _PatchedTileContext(nc) as tc:
        with (
            tc.tile_pool(name="whi", bufs=4) as whi_pool,
            tc.tile_pool(name="xhi", bufs=4) as xhi_pool,
            tc.tile_pool(name="wlo", bufs=3) as wlo_pool,
            tc.tile_pool(name="xlo", bufs=3) as xlo_pool,
            tc.tile_pool(name="consts", bufs=1) as consts,
            tc.tile_pool(name="signh", bufs=NJ * NBC) as signh_pool,
            tc.tile_pool(name="psum", bufs=8, space="PSUM") as psum_pool,
            tc.tile_pool(name="smx", bufs=4) as smx_pool,
        ):
            psumB = [
                [psum_pool.tile([128, 512], F32, name="psB", tag="psB")
                 for _ in range(NBC)]
                for _ in range(NJ)
            ]

            # warm16: fp16 so the warmup matmuls run in the same PE dtype
            # mode as the hi stream (no mode transition before real work).
            # inv128: the ones/128 stationary operand of the b2 fold.
            warm16 = consts.tile([128, 128], FP16, name="warm16", tag="warm16")
            nc.vector.memset(warm16[:], 0.0078125)
            inv128 = consts.tile([128, 128], BF16, name="inv128", tag="inv128")
            nc.vector.memset(inv128[:], 0.0078125)

            def hi_in(hq):
                w = whi_pool.tile([128, 4, H], FP16, name="whit", tag="whit")
                xf = xhi_pool.tile([128, 4, BC], FP16, name="xhit", tag="xhit")
                nc.sync.dma_start(w[:], whi[hq * 128:(hq + 1) * 128, :])
                nc.sync.dma_start(xf[:], xhi[hq * 128:(hq + 1) * 128, :])
                return w, xf

            def lo_in(g):
                w = wlo_pool.tile([128, 4, NJ, 2, 128], E5, name="wlot", tag="wlot")
                xf = xlo_pool.tile([128, 4, 2, BC], E4, name="xlot", tag="xlot")
                nc.sync.dma_start(w[:], wlo[g * 128:(g + 1) * 128, :])
                nc.sync.dma_start(xf[:], xlo[g * 128:(g + 1) * 128, :])
                return w, xf

            def hi_mms(w, xf, start):
                for i in range(4):
                    for j in range(NJ):
                        for bc in range(NBC):
                            nc.tensor.matmul(
                                psumB[j][bc][:],
                                w[:, i, j * 128:(j + 1) * 128],
                                xf[:, i, bc * 512:(bc + 1) * 512],
                                start=(start and i == 0), stop=False,
                            )

            def lo_mms(w, xf, u, j, bc, stop):
                nc.tensor.matmul(
                    psumB[j][bc][:],
                    w[:, u, j],
                    xf[:, u, :, bc * 512:(bc + 1) * 512],
                    start=False, stop=stop, perf_mode=DRMODE,
                )

            # ---- startup: quad 0 lands as 6 small pieces, each its own tile
            # (dependency tracking is tile-granular) and the first four issue
            # from different engine queues in parallel (each DMA issue
            # costs ~0.6us of queue time).
            with tc.high_priority():
                w00 = consts.tile([128, H], FP16, name="w00", tag="w00")
                nc.sync.dma_start(w00[:], whi[0:128, 0:H])
                x00a = consts.tile([128, 512], FP16, name="x00a", tag="x00a")
                nc.scalar.dma_start(x00a[:], xhi[0:128, 0:512])
                x00b = consts.tile([128, 512], FP16, name="x00b", tag="x00b")
                nc.gpsimd.dma_start(x00b[:], xhi[0:128, 512:BC])
                w01 = consts.tile([128, H], FP16, name="w01", tag="w01")
                nc.sync.dma_start(w01[:], whi[0:128, H:2 * H])
                x01 = consts.tile([128, BC], FP16, name="x01", tag="x01")
                nc.scalar.dma_start(x01[:], xhi[0:128, BC:2 * BC])
                w23 = consts.tile([128, 2, H], FP16, name="w23", tag="w23")
                nc.sync.dma_start(w23[:], whi[0:128, 2 * H:4 * H])
                x23 = consts.tile([128, 2, BC], FP16, name="x23", tag="x23")
                nc.scalar.dma_start(x23[:], xhi[0:128, 2 * BC:4 * BC])
            cb = consts.tile([128, 116], mybir.dt.uint8, name="cb", tag="cb")
            nc.sync.dma_start(cb[:], cblob[:, :])

            # Warmup matmuls: keep the PE busy (HAM activity window) from
            # ~7.4us until the first data lands. N=128 so each wasted warmup
            # is cheap; they all target the first 128 cols of bank (0,0),
            # overwritten by the first real start=True matmul.
            for _ in range(NWARM):
                nc.tensor.matmul(
                    psumB[0][0][:, 0:128], warm16[:], warm16[:],
                    start=True, stop=True,
                )

            # bc-major so the first eight matmuls consume x00a fully before
            # the first x00b use — matches the DMA arrival order. Exactly ONE
            # start=True full-width matmul per PSUM bank: a second start on
            # the same bank clears the whole bank's has_written bits and
            # silently drops the first region's contribution (measured —
            # deterministic sign-flip corruption).
            for i in range(4):
                for bc in range(NBC):
                    for j in range(NJ):
                        if i == 0:
                            rhs = (x00a[:] if bc == 0 else x00b[:])
                            lhsT = w00[:, j * 128:(j + 1) * 128]
                        else:
                            rhs = (x01[:, bc * 512:(bc + 1) * 512] if i == 1
                                   else x23[:, i - 2, bc * 512:(bc + 1) * 512])
                            lhsT = (w01[:, j * 128:(j + 1) * 128] if i == 1
                                    else w23[:, i - 2, j * 128:(j + 1) * 128])
                        nc.tensor.matmul(
                            psumB[j][bc][:], lhsT, rhs,
                            start=(i == 0), stop=False,
                        )

            # ---- hi quads 1..7 back-to-back (no fp8 interleave), then the
            # first three lo groups. Exactly one fp16->fp8 transition.
            lg_w = [None] * NLG
            lg_x = [None] * NLG
            for hq in range(1, NHQ):
                w, xf = hi_in(hq)
                # prefetch lo groups while the hi stream runs
                if hq == 3:
                    lg_w[0], lg_x[0] = lo_in(0)
                elif hq == 5:
                    lg_w[1], lg_x[1] = lo_in(1)
                elif hq == 7:
                    lg_w[2], lg_x[2] = lo_in(2)
                hi_mms(w, xf, start=False)
            for g in range(NLG - 1):
                for u in range(4):
                    for j in range(NJ):
                        for bc in range(NBC):
                            lo_mms(lg_w[g], lg_x[g], u, j, bc, stop=False)

            # ---- phase 2: last lo group bank-major; sign/mm2/softmax ----
            # PE order: [lo-bc0 16mm][lo-bc1 j0 4mm][mm2-bc0 20mm]
            # [lo-bc1 j1-3 12mm][mm2-bc1] — mm2-bc0 slots in after bc0's
            # signs have finished (they run under lo-bc1-j0), so bc0's whole
            # softmax + output DMA overlaps the rest of lo-bc1, and the PE
            # never waits for a sign().
            wl, xl = lo_in(NLG - 1)
            signh = [[None] * NBC for _ in range(NJ)]

            def lo_bank(j, bc):
                for u in range(4):
                    lo_mms(wl, xl, u, j, bc, stop=(u == 3))
                s = signh_pool.tile([128, 512], BF16, name="signh",
                                    tag="signh")
                nc.scalar.sign(s[:], psumB[j][bc][:],
                               bias=cb[:, j * 4:(j + 1) * 4].bitcast(F32))
                signh[j][bc] = s

            # One shared output tile for both bc chunks: a single output DMA
            # at the end (one less 0.6us issue on the sync queue ahead of
            # the final transfer).
            es2 = smx_pool.tile([128, NBT, SMXW], F32, name="es2", tag="es2")

            def mm2_smx(bc):
                # (b2 - 64) rides each accumulation group as a 5th matmul:
                # sum_p (1/128) * rep[p, c] == b2[c] - 64, so ps2 holds
                # logits - 64 and no softmax max-reduction is needed:
                # one batched exp(logits - 64) straight off PSUM, per-bt
                # sums via one segmented 3D reduce; host divides by the sum.
                # (Fusing the sum into ACT accum_out measured worse: the
                # ACTIVATION_READ_ACCUMULATOR costs ~300ns per bt and
                # serializes 8 chains on the ACT queue.)
                ps2 = psum_pool.tile([128, 4, C], F32, name="psD", tag="psB")
                for t in range(4):
                    nc.tensor.matmul(
                        ps2[:, t], inv128[:],
                        cb[:, 96:116].bitcast(BF16),
                        start=True, stop=False,
                    )
                    for j in range(NJ):
                        nc.tensor.matmul(
                            ps2[:, t],
                            signh[j][bc][:, t * 128:(t + 1) * 128],
                            cb[:, 16 + j * 20:16 + (j + 1) * 20].bitcast(BF16),
                            start=False,
                            stop=(j == NJ - 1),
                        )
                es = es2[:, bc * 4:(bc + 1) * 4]
                nc.scalar.activation(
                    es[:, :, 0:C], ps2[:], mybir.ActivationFunctionType.Exp,
                )
                nc.vector.reduce_sum(es[:, :, C:C + 1], es[:, :, 0:C],
                                     axis=mybir.AxisListType.X)

            for bc in range(NBC):
                for j in range(NJ):
                    lo_bank(j, bc)
            mm2_smx(0)
            mm2_smx(1)
            nc.sync.dma_start(out[:, :], es2[:])

    _install_wait_splitter(nc)
    return nc


_cached_nc = None


def _get_nc() -> bass.Bass:
    global _cached_nc
    if _cached_nc is None:
        _cached_nc = build_kernel()
    return _cached_nc


def kernel(inputs, W1, b1, W2, b2):
    e4 = ml_dtypes.float8_e4m3
    e5 = ml_dtypes.float8_e5m2
    x = np.ascontiguousarray(np.asarray(inputs, dtype=np.float32))
    W1 = np.asarray(W1, dtype=np.float32)
    b1 = np.asarray(b1, dtype=np.float32)
    W2 = np.asarray(W2, dtype=np.float32)
    b2 = np.asarray(b2, dtype=np.float32)

    S1 = np.where(W1 >= 0, 1.0, -1.0).astype(np.float32)  # [F, H]
    # hi weights: [hq, i, 128p, H] -> [hq*128+p, i*H + jcol]
    whi_pack = np.ascontiguousarray(
        S1.astype(np.float16)
        .reshape(NHQ, 4, 128, H)
        .transpose(0, 2, 1, 3)
        .reshape(NHQ * 128, 4 * H)
    )
    # lo weights: f = (g*4+u)*256 + k*128 + p
    wlo_t = (S1 * (2.0 ** -LOSHIFT)).astype(e5)
    wlo_pack = np.ascontiguousarray(
        wlo_t.reshape(NLG, 4, 2, 128, NJ, 128)
        .transpose(0, 3, 1, 4, 2, 5)
        .reshape(NLG * 128, 4 * NJ * 256)
    )
    b1_pack = np.ascontiguousarray(b1.reshape(NJ, 128).T)
    S2w = np.where(W2 >= 0, 1.0, -1.0)
    w2_pack = np.ascontiguousarray(
        S2w.reshape(NJ, 128, C).transpose(1, 0, 2).reshape(128, NJ * C)
    ).astype(ml_dtypes.bfloat16)
    b2_rep = np.ascontiguousarray(
        np.broadcast_to((b2 - 64.0).reshape(1, C), (128, C))
    ).astype(ml_dtypes.bfloat16)
    cblob_pack = np.ascontiguousarray(np.concatenate([
        b1_pack.astype(np.float32).view(np.uint8),
        w2_pack.view(np.uint8),
        b2_rep.view(np.uint8),
    ], axis=1))
    assert cblob_pack.shape == (128, 116)

    in_maps = []
    for c in range(NCORES):
        xc_t = np.ascontiguousarray(x[c * BC:(c + 1) * BC, :].T)  # [F, BC]
        hi = xc_t.astype(np.float16)
        lo8 = ((xc_t - hi.astype(np.float32)) * (2.0 ** LOSHIFT)).astype(e4)
        xhi_pack = np.ascontiguousarray(
            hi.reshape(NHQ, 4, 128, BC).transpose(0, 2, 1, 3)
            .reshape(NHQ * 128, 4 * BC)
        )
        xlo_pack = np.ascontiguousarray(
            lo8.reshape(NLG, 4, 2, 128, BC).transpose(0, 3, 1, 2, 4)
            .reshape(NLG * 128, 4 * 2 * BC)
        )
        in_maps.append(
            {
                "xhi": xhi_pack,
                "whi": whi_pack,
                "xlo": xlo_pack,
                "wlo": wlo_pack,
                "cblob": cblob_pack,
            }
        )

    nc = _get_nc()
    res = run_bass_kernel_spmd(nc, in_maps, core_ids=list(range(NCORES)))
    global last_results
    last_results = res
    parts = []
    for c in range(NCORES):
        oc = res.results[c]["out"]  # [128, NBC*4*SMXW]
        es = oc.reshape(128, NBT, SMXW)
        probs = es[:, :, 0:C] / es[:, :, C:C + 1]  # exp / sum
        parts.append(probs.transpose(1, 0, 2).reshape(BC, C))
    return np.concatenate(parts, axis=0).astype(np.float32)


last_results = None


# revision 36
# speedup vs baseline: 1.0103x; 1.0103x over previous
"""Binarized-MLP (BNN) kernel for Trainium2, data-parallel over batch on 8 cores.

Reference computation:
    h      = x @ sign(W1) + b1          x:[8192,4096] W1:[4096,512]
    logits = sign(h) @ sign(W2) + b2    W2:[512,10]
    out    = softmax(logits)            [8192,10]

Strategy (per core, batch shard of 1024 rows):
  - x is host-split as x = hi + lo/2^11 with hi = fp16(x) (11 significant
    bits) and lo = e4m3((x - hi) * 2^11). The hi pass runs as 256 normal
    fp16 matmuls (K=128 stationary tiles); the lo pass runs as 128 fp8
    DoubleRow matmuls (K=256 tiles, 2 MACs/cell/cycle). The lo scale 2^-11
    is folded into the stationary operand as sign(W1) * 2^-11 in e5m2.
    Total stream: 384 matmuls = 1.5x one bf16 copy, h accurate to ~2e-4.
  - Stream order: hi quads 0..7 back-to-back, then lo groups 0..3, then
    the bf16 second layer — exactly 2 PE dtype-mode transitions
    (fp16->fp8, fp8->bf16), each ~0.2-0.6us.
  - DMA rings: the steady stream (whi, xhi, wlo, xlo, output) rides the
    sync HWDGE ring, whose FIFO gives just-in-time delivery order.
    Startup pieces x00a/x01/x23 ride the scalar (ACT) ring and x00b the
    gpsimd SWDGE ring for parallel first-transfer latency. (Moving the
    steady stream onto the scalar ring silently corrupts data beyond ~3
    queued transfers; eager xlo prefetch on a second ring also steals
    early HBM bandwidth from the critical quad-1 path — both measured.)
  - Startup: N=128 fp16 warmup matmuls off a memset tile bridge the PE
    from ~7.4us until the first data lands ~11us, keeping the HAM
    activity window busy so the PE reaches 2.4GHz ~3.4us after the first
    warmup with no mid-ramp resets (a >150ns gap during the ramp resets
    the 4096-cycle window and costs ~2.8us).
  - PSUM: 8 banks [j][bc] of [128,512] accumulate all 48 matmuls per bank.
  - Phase 2: last lo group bank-major; all 8 sign() activations are
    emitted before any softmax work (strict-FIFO ACT queue). b2 - 64 is
    folded into the second matmul's accumulation group as a 5th matmul
    (stationary = ones/128 in bf16, moving = (b2-64) replicated), so ps2
    holds logits - 64 and softmax needs NO max-reduction: max|logit| ~
    6.7 sigma = 150 << 88+64, so exp(logit-64) cannot overflow fp32, and
    exp underflow of all 10 classes needs max < -23 (P ~ 1e-8/row).
    Per 4-bt chunk: one ACT Exp straight off PSUM -> segmented
    reduce_sum -> reciprocal, exp and 1/sum ship to the host which does
    the broadcast multiply (a dequant-like pointwise scale).
  - Output: packed [p, bt*11+c] f32 (10 exp values + sum per bt), one
    DMA on the sync ring; host multiplies and reorders to [1024,10].
"""

import numpy as np
import ml_dtypes

import concourse.bass as bass
import concourse.tile as tile
from concourse import mybir
from concourse.bass_utils import run_bass_kernel_spmd
from bass_rust import ScopedClock, VectorClock

_CLEAR_SEMS = True

E4 = mybir.dt.float8e4
E5 = mybir.dt.float8e5
FP16 = mybir.dt.float16
BF16 = mybir.dt.bfloat16
F32 = mybir.dt.float32
DRMODE = mybir.MatmulPerfMode.DoubleRow

B, F, H, C = 8192, 4096, 512, 10
NCORES = 8
BC = B // NCORES          # 1024 batch rows per core
NFT = F // 128            # 32 hi f-tiles
NHQ = NFT // 4            # 8 hi quads
NQ = F // 256             # 16 lo DR blocks
NLG = NQ // 4             # 4 lo groups
NJ = H // 128             # 4 j-tiles
NBC = BC // 512           # 2 moving chunks of 512
NBT = BC // 128           # 8 output b-tiles
LOSHIFT = 11              # lo scale 2^11
NWARM = 33                # N=128 warmup matmuls bridging DMA latency
SMXW = C + 1              # per-bt output: 10 exp values + 1 row sum


class _PatchedTileContext(tile.TileContext):
    """Workaround for the walrus build in this container only accepting one
    sem wait on a CTRL-type (Drain) instruction: spread the exit drain's
    per-proc waits across several drains with one wait each."""

    def _drain_and_barrier(self, tick_clock, wait_clock):
        gc = tick_clock.global_clock
        ticks = list(gc)
        nprocs = len(ticks)
        engines = [
            self.nc.sync,
            self.nc.gpsimd,
            self.nc.vector,
            self.nc.scalar,
            self.nc.tensor,
        ]
        k = 0
        for i, t in enumerate(ticks):
            if t == 0:
                continue
            partial = [0] * nprocs
            partial[i] = t
            inst = engines[k % len(engines)].nop()
            k += 1
            wait_clock.add_sem_waits(
                inst.ins, ScopedClock({None: VectorClock(partial)})
            )
        self.nc.sync.drain()

        self.nc.all_engine_barrier(sem_only=True)
        assert self.sems is not None
        popped = self.nc._tile_sem_poison_stack.pop()
        assert popped is self._sem_poison
        if _CLEAR_SEMS:
            self.nc.clear_and_free_semaphores(list(self.sems.allocated().values()))


def _split_waits_json(raw: bytes) -> bytes:
    """The walrus build in this container accepts at most ONE sem wait per
    instruction. Rewrite the serialized BIR: excess waits become standalone
    EventSemaphore wait instructions on the same engine immediately before
    the instruction."""
    import json as _json

    m = _json.loads(raw)
    ctr = 0
    for fn in m.get("functions", []):
        for bb in fn.get("blocks", []):
            insts = bb.get("instructions", [])
            new_insts = []
            for inst in insts:
                si = inst.get("sync_info")
                waits = si.get("on_wait") or [] if si else []
                if len(waits) > 1:
                    for w in waits[:-1]:
                        new_insts.append(
                            {
                                "debug": inst.get("debug", 0),
                                "engine": inst["engine"],
                                "ins": [],
                                "outs": [],
                                "name": f"WSPLIT-{ctr}",
                                "opcode": "EventSemaphore",
                                "sync_info": {"on_update": [], "on_wait": [w]},
                            }
                        )
                        ctr += 1
                    si["on_wait"] = [waits[-1]]
                new_insts.append(inst)
            bb["instructions"] = new_insts
    return _json.dumps(m).encode()


def _install_wait_splitter(nc: bass.Bass) -> None:
    orig = nc.to_json_bytes

    def patched():
        return _split_waits_json(orig())

    nc.to_json_bytes = patched


def build_kernel() -> bass.Bass:
    nc = bass.Bass()
    # hi stream: row hq*128+p, free [i=0..3][b]   (8KB / partition line)
    xhi = nc.dram_tensor("xhi", [NHQ * 128, 4 * BC], FP16, kind="ExternalInput")
    # hi weights: row hq*128+p, free [i=0..3][j*128+col]  (4KB / line)
    whi = nc.dram_tensor("whi", [NHQ * 128, 4 * H], FP16, kind="ExternalInput")
    # lo stream: row g*128+p, free [u=0..3][k][b]  (8KB / line)
    xlo = nc.dram_tensor("xlo", [NLG * 128, 4 * 2 * BC], E4, kind="ExternalInput")
    # lo weights: row g*128+p, free [u][j][k][col]  (4KB / line)
    wlo = nc.dram_tensor("wlo", [NLG * 128, 4 * NJ * 256], E5, kind="ExternalInput")
    # constants packed as one byte blob: b1 f32 [0:16) | sign(W2) bf16
    # [16:96) | (b2-64) bf16 [96:116)
    cblob = nc.dram_tensor("cblob", [128, 116], mybir.dt.uint8,
                           kind="ExternalInput")
    out = nc.dram_tensor("out", [128, NBC * 4 * SMXW], F32, kind="ExternalOutput")

    with _PatchedTileContext(nc) as tc:
        with (
            tc.tile_pool(name="whi", bufs=4) as whi_pool,
            tc.tile_pool(name="xhi", bufs=4) as xhi_pool,
            tc.tile_pool(name="wlo", bufs=3) as wlo_pool,
            tc.tile_pool(name="xlo", bufs=3) as xlo_pool,
            tc.tile_pool(name="consts", bufs=1) as consts,
            tc.tile_pool(name="signh", bufs=NJ * NBC) as signh_pool,
            tc.tile_pool(name="psum", bufs=8, space="PSUM") as psum_pool,
            tc.tile_pool(name="smx", bufs=4) as smx_pool,
        ):
            psumB = [
                [psum_pool.tile([128, 512], F32, name="psB", tag="psB")
                 for _ in range(NBC)]
                for _ in range(NJ)
            ]

            # warm16: fp16 so the warmup matmuls run in the same PE dtype
            # mode as the hi stream (no mode transition before real work).
            # inv128: the ones/128 stationary operand of the b2 fold.
            warm16 = consts.tile([128, 128], FP16, name="warm16", tag="warm16")
            nc.vector.memset(warm16[:], 0.0078125)
            inv128 = consts.tile([128, 128], BF16, name="inv128", tag="inv128")
            nc.vector.memset(inv128[:], 0.0078125)

            def hi_in(hq):
                # Each piece as TWO DMAs: consecutive DMAs land on different
                # HWDGE queues (round-robin) and transfer in parallel, so a
                # piece's latency is halved — a single queue moves only
                # ~70-90GB/s while the ring aggregates ~200GB/s.
                w = whi_pool.tile([128, 4, H], FP16, name="whit", tag="whit")
                xf = xhi_pool.tile([128, 4, BC], FP16, name="xhit", tag="xhit")
                row_w = whi[hq * 128:(hq + 1) * 128, :]
                row_x = xhi[hq * 128:(hq + 1) * 128, :]
                nc.sync.dma_start(w[:, 0:2], row_w[:, 0:2 * H])
                nc.sync.dma_start(w[:, 2:4], row_w[:, 2 * H:4 * H])
                nc.sync.dma_start(xf[:, 0:2], row_x[:, 0:2 * BC])
                nc.sync.dma_start(xf[:, 2:4], row_x[:, 2 * BC:4 * BC])
                return w, xf

            def lo_in(g):
                w = wlo_pool.tile([128, 4, NJ, 2, 128], E5, name="wlot", tag="wlot")
                xf = xlo_pool.tile([128, 4, 2, BC], E4, name="xlot", tag="xlot")
                nc.sync.dma_start(w[:], wlo[g * 128:(g + 1) * 128, :])
                nc.sync.dma_start(xf[:], xlo[g * 128:(g + 1) * 128, :])
                return w, xf

            def hi_mms(w, xf, start):
                for i in range(4):
                    for j in range(NJ):
                        for bc in range(NBC):
                            nc.tensor.matmul(
                                psumB[j][bc][:],
                                w[:, i, j * 128:(j + 1) * 128],
                                xf[:, i, bc * 512:(bc + 1) * 512],
                                start=(start and i == 0), stop=False,
                            )

            def lo_mms(w, xf, u, j, bc, stop):
                nc.tensor.matmul(
                    psumB[j][bc][:],
                    w[:, u, j],
                    xf[:, u, :, bc * 512:(bc + 1) * 512],
                    start=False, stop=stop, perf_mode=DRMODE,
                )

            # ---- startup: quad 0 lands as 6 small pieces, each its own tile
            # (dependency tracking is tile-granular) and the first four issue
            # from different engine queues in parallel (each DMA issue
            # costs ~0.6us of queue time).
            with tc.high_priority():
                # first pieces as 2 DMAs each: halves ride different HWDGE
                # queues in parallel (halves the first-data latency + jitter)
                w00 = consts.tile([128, H], FP16, name="w00", tag="w00")
                nc.sync.dma_start(w00[:, 0:256], whi[0:128, 0:256])
                nc.sync.dma_start(w00[:, 256:H], whi[0:128, 256:H])
                x00a = consts.tile([128, 512], FP16, name="x00a", tag="x00a")
                nc.scalar.dma_start(x00a[:, 0:256], xhi[0:128, 0:256])
                nc.scalar.dma_start(x00a[:, 256:512], xhi[0:128, 256:512])
                x00b = consts.tile([128, 512], FP16, name="x00b", tag="x00b")
                nc.gpsimd.dma_start(x00b[:], xhi[0:128, 512:BC])
                w01 = consts.tile([128, H], FP16, name="w01", tag="w01")
                nc.sync.dma_start(w01[:], whi[0:128, H:2 * H])
                x01 = consts.tile([128, BC], FP16, name="x01", tag="x01")
                nc.scalar.dma_start(x01[:], xhi[0:128, BC:2 * BC])
                w23 = consts.tile([128, 2, H], FP16, name="w23", tag="w23")
                nc.sync.dma_start(w23[:], whi[0:128, 2 * H:4 * H])
                x23 = consts.tile([128, 2, BC], FP16, name="x23", tag="x23")
                nc.scalar.dma_start(x23[:], xhi[0:128, 2 * BC:4 * BC])
            cb = consts.tile([128, 116], mybir.dt.uint8, name="cb", tag="cb")
            nc.sync.dma_start(cb[:], cblob[:, :])

            # Warmup matmuls: keep the PE busy (HAM activity window) from
            # ~7.4us until the first data lands. N=128 so each wasted warmup
            # is cheap; they all target the first 128 cols of bank (0,0),
            # overwritten by the first real start=True matmul.
            for _ in range(NWARM):
                nc.tensor.matmul(
                    psumB[0][0][:, 0:128], warm16[:], warm16[:],
                    start=True, stop=True,
                )

            # bc-major so the first eight matmuls consume x00a fully before
            # the first x00b use — matches the DMA arrival order. Exactly ONE
            # start=True full-width matmul per PSUM bank: a second start on
            # the same bank clears the whole bank's has_written bits and
            # silently drops the first region's contribution (measured —
            # deterministic sign-flip corruption).
            for i in range(4):
                for bc in range(NBC):
                    for j in range(NJ):
                        if i == 0:
                            rhs = (x00a[:] if bc == 0 else x00b[:])
                            lhsT = w00[:, j * 128:(j + 1) * 128]
                        else:
                            rhs = (x01[:, bc * 512:(bc + 1) * 512] if i == 1
                                   else x23[:, i - 2, bc * 512:(bc + 1) * 512])
                            lhsT = (w01[:, j * 128:(j + 1) * 128] if i == 1
                                    else w23[:, i - 2, j * 128:(j + 1) * 128])
                        nc.tensor.matmul(
                            psumB[j][bc][:], lhsT, rhs,
                            start=(i == 0), stop=False,
                        )

            # ---- hi quads 1..7 back-to-back (no fp8 interleave), then the
            # first three lo groups. Exactly one fp16->fp8 transition.
            lg_w = [None] * NLG
            lg_x = [None] * NLG
            for hq in range(1, NHQ):
                w, xf = hi_in(hq)
                # prefetch lo groups while the hi stream runs
                if hq == 3:
                    lg_w[0], lg_x[0] = lo_in(0)
                elif hq == 5:
                    lg_w[1], lg_x[1] = lo_in(1)
                elif hq == 7:
                    lg_w[2], lg_x[2] = lo_in(2)
                hi_mms(w, xf, start=False)
            for g in range(NLG - 1):
                for u in range(4):
                    for j in range(NJ):
                        for bc in range(NBC):
                            lo_mms(lg_w[g], lg_x[g], u, j, bc, stop=False)

            # ---- phase 2: last lo group bank-major; sign/mm2/softmax ----
            # PE order: [lo-bc0 16mm][lo-bc1 j0 4mm][mm2-bc0 20mm]
            # [lo-bc1 j1-3 12mm][mm2-bc1] — mm2-bc0 slots in after bc0's
            # signs have finished (they run under lo-bc1-j0), so bc0's whole
            # softmax + output DMA overlaps the rest of lo-bc1, and the PE
            # never waits for a sign().
            wl, xl = lo_in(NLG - 1)
            signh = [[None] * NBC for _ in range(NJ)]

            def lo_bank(j, bc):
                for u in range(4):
                    lo_mms(wl, xl, u, j, bc, stop=(u == 3))
                s = signh_pool.tile([128, 512], BF16, name="signh",
                                    tag="signh")
                nc.scalar.sign(s[:], psumB[j][bc][:],
                               bias=cb[:, j * 4:(j + 1) * 4].bitcast(F32))
                signh[j][bc] = s

            # One shared output tile for both bc chunks: a single output DMA
            # at the end (one less 0.6us issue on the sync queue ahead of
            # the final transfer).
            es2 = smx_pool.tile([128, NBT, SMXW], F32, name="es2", tag="es2")

            def mm2_smx(bc):
                # (b2 - 64) rides each accumulation group as a 5th matmul:
                # sum_p (1/128) * rep[p, c] == b2[c] - 64, so ps2 holds
                # logits - 64 and no softmax max-reduction is needed:
                # one batched exp(logits - 64) straight off PSUM, per-bt
                # sums via one segmented 3D reduce; host divides by the sum.
                # (Fusing the sum into ACT accum_out measured worse: the
                # ACTIVATION_READ_ACCUMULATOR costs ~300ns per bt and
                # serializes 8 chains on the ACT queue.)
                ps2 = psum_pool.tile([128, 4, C], F32, name="psD", tag="psB")
                for t in range(4):
                    nc.tensor.matmul(
                        ps2[:, t], inv128[:],
                        cb[:, 96:116].bitcast(BF16),
                        start=True, stop=False,
                    )
                    for j in range(NJ):
                        nc.tensor.matmul(
                            ps2[:, t],
                            signh[j][bc][:, t * 128:(t + 1) * 128],
                            cb[:, 16 + j * 20:16 + (j + 1) * 20].bitcast(BF16),
                            start=False,
                            stop=(j == NJ - 1),
                        )
                es = es2[:, bc * 4:(bc + 1) * 4]
                nc.scalar.activation(
                    es[:, :, 0:C], ps2[:], mybir.ActivationFunctionType.Exp,
                )
                nc.vector.reduce_sum(es[:, :, C:C + 1], es[:, :, 0:C],
                                     axis=mybir.AxisListType.X)

            for bc in range(NBC):
                for j in range(NJ):
                    lo_bank(j, bc)
            mm2_smx(0)
            mm2_smx(1)
            nc.sync.dma_start(out[:, :], es2[:])

    _install_wait_splitter(nc)
    return nc


_cached_nc = None


def _get_nc() -> bass.Bass:
    global _cached_nc
    if _cached_nc is None:
        _cached_nc = build_kernel()
    return _cached_nc


def kernel(inputs, W1, b1, W2, b2):
    e4 = ml_dtypes.float8_e4m3
    e5 = ml_dtypes.float8_e5m2
    x = np.ascontiguousarray(np.asarray(inputs, dtype=np.float32))
    W1 = np.asarray(W1, dtype=np.float32)
    b1 = np.asarray(b1, dtype=np.float32)
    W2 = np.asarray(W2, dtype=np.float32)
    b2 = np.asarray(b2, dtype=np.float32)

    S1 = np.where(W1 >= 0, 1.0, -1.0).astype(np.float32)  # [F, H]
    # hi weights: [hq, i, 128p, H] -> [hq*128+p, i*H + jcol]
    whi_pack = np.ascontiguousarray(
        S1.astype(np.float16)
        .reshape(NHQ, 4, 128, H)
        .transpose(0, 2, 1, 3)
        .reshape(NHQ * 128, 4 * H)
    )
    # lo weights: f = (g*4+u)*256 + k*128 + p
    wlo_t = (S1 * (2.0 ** -LOSHIFT)).astype(e5)
    wlo_pack = np.ascontiguousarray(
        wlo_t.reshape(NLG, 4, 2, 128, NJ, 128)
        .transpose(0, 3, 1, 4, 2, 5)
        .reshape(NLG * 128, 4 * NJ * 256)
    )
    b1_pack = np.ascontiguousarray(b1.reshape(NJ, 128).T)
    S2w = np.where(W2 >= 0, 1.0, -1.0)
    w2_pack = np.ascontiguousarray(
        S2w.reshape(NJ, 128, C).transpose(1, 0, 2).reshape(128, NJ * C)
    ).astype(ml_dtypes.bfloat16)
    b2_rep = np.ascontiguousarray(
        np.broadcast_to((b2 - 64.0).reshape(1, C), (128, C))
    ).astype(ml_dtypes.bfloat16)
    cblob_pack = np.ascontiguousarray(np.concatenate([
        b1_pack.astype(np.float32).view(np.uint8),
        w2_pack.view(np.uint8),
        b2_rep.view(np.uint8),
    ], axis=1))
    assert cblob_pack.shape == (128, 116)

    in_maps = []
    for c in range(NCORES):
        xc_t = np.ascontiguousarray(x[c * BC:(c + 1) * BC, :].T)  # [F, BC]
        hi = xc_t.astype(np.float16)
        lo8 = ((xc_t - hi.astype(np.float32)) * (2.0 ** LOSHIFT)).astype(e4)
        xhi_pack = np.ascontiguousarray(
            hi.reshape(NHQ, 4, 128, BC).transpose(0, 2, 1, 3)
            .reshape(NHQ * 128, 4 * BC)
        )
        xlo_pack = np.ascontiguousarray(
            lo8.reshape(NLG, 4, 2, 128, BC).transpose(0, 3, 1, 2, 4)
            .reshape(NLG * 128, 4 * 2 * BC)
        )
        in_maps.append(
            {
                "xhi": xhi_pack,
                "whi": whi_pack,
                "xlo": xlo_pack,
                "wlo": wlo_pack,
                "cblob": cblob_pack,
            }
        )

    nc = _get_nc()
    res = run_bass_kernel_spmd(nc, in_maps, core_ids=list(range(NCORES)))
    global last_results
    last_results = res
    parts = []
    for c in range(NCORES):
        oc = res.results[c]["out"]  # [128, NBC*4*SMXW]
        es = oc.reshape(128, NBT, SMXW)
        probs = es[:, :, 0:C] / es[:, :, C:C + 1]  # exp / sum
        parts.append(probs.transpose(1, 0, 2).reshape(BC, C))
    return np.concatenate(parts, axis=0).astype(np.float32)


last_results = None
